# revision 42
# baseline (speedup 1.0000x reference)
"""Trainium2 Bass kernel for CausalSelfAttention with KV-prefix cache.

Problem (hardcoded): B=2, T=2048, C=1024, H=16, D=64, P=2048.
Sharding: 8 cores = 2 (batch) x 4 (head groups of 4 heads).
Each core computes, for its (b, 4 heads):
  qkv slice -> prefix+causal softmax -> AV -> partial W_proj product.
Host sums the 4 per-core partial projections per batch and transposes.

All attention math runs in a transposed layout (features/keys on the
partition dim) so no on-device transposes are needed:
  x^T [C,T] -> q^T,k^T (pair-packed [128,T]) via lhsT=W_attn slice
  S^T [keys, tq] via row-tiled (2 heads) K=64 matmuls
  exp on ScalarE (PSUM->SBUF, bf16, fused scale=1/sqrt(D))
  AV per head with an appended ones-column (M=65) so the softmax
  denominator accumulates for free in PSUM partition 64
  per-query normalization via reciprocal + K=65 broadcast matmul
  out^T = W_proj_slice^T @ y^T  (per-core partial)

Phase 2 is software-pipelined: score matmuls for chunk c+1 are issued
before the AV matmuls of chunk c, so the ScalarE exp (the per-chunk
serial dependency) overlaps the TensorE work of neighboring chunks.
"""

import numpy as np
import ml_dtypes
from contextlib import ExitStack

import concourse.bacc as bacc
import concourse.tile as tile
import concourse.mybir as mybir
from concourse.bass_utils import run_bass_kernel_spmd

F32 = mybir.dt.float32
F32R = mybir.dt.float32r
BF16 = mybir.dt.bfloat16
I16 = mybir.dt.int16
EXP = mybir.ActivationFunctionType.Exp
COPY = mybir.ActivationFunctionType.Copy
MULT = mybir.AluOpType.mult
ADD = mybir.AluOpType.add

B, T, C, H, D, P = 2, 2048, 1024, 16, 64, 2048
HPC = 4            # heads per core
NPAIR = 2          # head pairs per core
TQ = 512           # query block (matmul free dim)
KC = 128           # key chunk (PSUM partition dim)
SCALE = 1.0 / np.sqrt(D)
# Schraudolph bf16 exp-approx constants: exp(x*SCALE) ~=
# bitcast_bf16(int16(SCH_A*x + SCH_B)); DVE rounds to nearest.
SCH_A = 128.0 / np.log(2.0) * SCALE
SCH_B = 127.0 * 128.0 - 7.0


def build_kernel(t=T, p=P, c=C, n_cores=8):
    """Build + compile the SPMD Bass program. Same program on every core."""
    nt = t // TQ            # query blocks
    npc = p // KC           # prefix key chunks
    nck = c // 128          # C contraction chunks
    ntc = t // 128          # T rows in 128-chunks

    nc = bacc.Bacc("TRN2", target_bir_lowering=False, debug=False,
                   num_devices=n_cores)

    x_t = nc.dram_tensor("x_t", [c, t], BF16, kind="ExternalInput").ap()
    w_qk = nc.dram_tensor("w_qk", [c, 2 * HPC * D], BF16, kind="ExternalInput").ap()
    w_v = nc.dram_tensor("w_v", [c, HPC * D], BF16, kind="ExternalInput").ap()
    ckt = nc.dram_tensor("ckt", [NPAIR, 128, p], BF16, kind="ExternalInput").ap()
    cv = nc.dram_tensor("cv", [NPAIR, 128, npc, 2, 65], BF16, kind="ExternalInput").ap()
    wp = nc.dram_tensor("wp", [NPAIR, 128, c], BF16, kind="ExternalInput").ap()
    masks = nc.dram_tensor("masks", [128, 4, TQ], BF16, kind="ExternalInput").ap()
    bsel = nc.dram_tensor("bsel", [65, 128], F32, kind="ExternalInput").ap()
    zrd = nc.dram_tensor("zrd", [65, 2 * 4, TQ], F32, kind="ExternalInput").ap()
    out_t = nc.dram_tensor("out_t", [c, t], BF16, kind="ExternalOutput").ap()

    with tile.TileContext(nc) as tc, ExitStack() as top:
        const = top.enter_context(tc.tile_pool(name="const", bufs=1))
        persist = top.enter_context(tc.tile_pool(name="persist", bufs=1))

        # ---- persistent SBUF tensors -------------------------------------
        qT = [persist.tile([128, t], BF16, tag=f"qT{i}", name=f"qT{i}") for i in range(NPAIR)]
        kT = [persist.tile([128, t], BF16, tag=f"kT{i}", name=f"kT{i}") for i in range(NPAIR)]
        cktT = [persist.tile([128, p], BF16, tag=f"cktT{i}", name=f"cktT{i}") for i in range(NPAIR)]
        # v with ones column, [.., eo, pair, 65]: even heads [v|1] (denom at
        # PSUM partition 64), odd heads [1|v] (denom at partition 0)
        vt = persist.tile([128, ntc, 2, NPAIR, 65], BF16, tag="vt", name="vt")
        cvt = [persist.tile([128, npc, 2, 65], BF16, tag=f"cvt{i}", name=f"cvt{i}") for i in range(NPAIR)]
        wpt = [persist.tile([128, c], BF16, tag=f"wpt{i}", name=f"wpt{i}") for i in range(NPAIR)]
        maskt = const.tile([128, 4, TQ], BF16, tag="maskt", name="maskt")
        bselt = const.tile([65, 128], F32R, tag="bselt", name="bselt")
        ysb = [persist.tile([128, nt, TQ], BF16, tag=f"ysb{i}", name=f"ysb{i}") for i in range(NPAIR)]
        # reciprocal rows live at partitions 64 (even head) / 0 (odd head);
        # one slot per (pair, tb) since normalization is deferred to phase 3
        rdc = persist.tile([65, NPAIR * nt, TQ], F32R, tag="rdc", name="rdc")

        nc.gpsimd.memset(vt[:, :, 0, :, 64:65], 1.0)
        nc.gpsimd.memset(vt[:, :, 1, :, 0:1], 1.0)

        # ---- phase 1: QKV projection (x streamed by query block) ----------
        with ExitStack() as ph1:
            xin = ph1.enter_context(tc.tile_pool(name="xin", bufs=2))
            win = ph1.enter_context(tc.tile_pool(name="win", bufs=1))
            ps_qk = ph1.enter_context(tc.tile_pool(name="ps_qk", bufs=2, space="PSUM"))
            ps_v = ph1.enter_context(tc.tile_pool(name="ps_v", bufs=2, space="PSUM"))

            wqkt = win.tile([128, nck, 2 * HPC * D], BF16, tag="wqkt", name="wqkt")
            wvt = win.tile([128, nck, HPC * D], BF16, tag="wvt", name="wvt")
            # DMA order: get the first qk chain's operands in ASAP
            xt0 = xin.tile([128, nck, TQ], BF16, tag="xt", name="xt")
            for kc_ in range(nck):
                nc.sync.dma_start(wqkt[:, kc_, :], w_qk[kc_ * 128:(kc_ + 1) * 128, :])
                nc.sync.dma_start(xt0[:, kc_, :], x_t[kc_ * 128:(kc_ + 1) * 128, 0:TQ])
            for kc_ in range(nck):
                nc.sync.dma_start(wvt[:, kc_, :], w_v[kc_ * 128:(kc_ + 1) * 128, :])
            nc.sync.dma_start(maskt[:], masks[:, :, :])
            nc.sync.dma_start(bselt[:], bsel[:, :].bitcast(F32R))
            nc.sync.dma_start(rdc[:], zrd[:].bitcast(F32R))

            for nb in range(nt):
                if nb == 0:
                    xt = xt0
                else:
                    xt = xin.tile([128, nck, TQ], BF16, tag="xt", name="xt")
                    for kc_ in range(nck):
                        nc.sync.dma_start(
                            xt[:, kc_, :],
                            x_t[kc_ * 128:(kc_ + 1) * 128, nb * TQ:(nb + 1) * TQ])
                # q^T / k^T: out chunk mc (128 rows = one head pair of q or k)
                for mc in range(4):
                    dest = qT[mc] if mc < 2 else kT[mc - 2]
                    ps = ps_qk.tile([128, TQ], F32, tag="ps_qk", name="psqk")
                    for kc_ in range(nck):
                        nc.tensor.matmul(
                            ps[:],
                            wqkt[:, kc_, mc * 128:(mc + 1) * 128],
                            xt[:, kc_, :],
                            start=(kc_ == 0), stop=(kc_ == nck - 1),
                        )
                    nc.scalar.activation(dest[:, nb * TQ:(nb + 1) * TQ], ps[:], COPY)
                # v in natural layout [t, (eo, pair), 64]; w_v columns are
                # host-ordered evens-first so each evac is one strided copy
                for tcl in range(TQ // 128):
                    tc_ = nb * (TQ // 128) + tcl
                    ps = ps_v.tile([128, 2, NPAIR, D], F32, tag="ps_v", name="psv")
                    for kc_ in range(nck):
                        nc.tensor.matmul(
                            ps[:, :, :, :],
                            xt[:, kc_, tcl * 128:(tcl + 1) * 128],
                            wvt[:, kc_, :],
                            start=(kc_ == 0), stop=(kc_ == nck - 1),
                        )
                    nc.scalar.activation(vt[:, tc_, 0, :, 0:64], ps[:, 0, :, :], COPY)
                    nc.scalar.activation(vt[:, tc_, 1, :, 1:65], ps[:, 1, :, :], COPY)

        # KV-cache (needed from phase 2 on; DMA'd behind x)
        for i in range(NPAIR):
            nc.sync.dma_start(cktT[i][:], ckt[i, :, :])
            nc.sync.dma_start(cvt[i][:], cv[i, :, :, :, :])

        # ---- phase 2: attention (software-pipelined) ---------------------
        with ExitStack() as ph2:
            sbp = ph2.enter_context(tc.tile_pool(name="sbp", bufs=2, space="PSUM"))
            ybp = ph2.enter_context(tc.tile_pool(name="ybp", bufs=2, space="PSUM"))
            ebp = ph2.enter_context(tc.tile_pool(name="ebp", bufs=3))
            stp = ph2.enter_context(tc.tile_pool(name="stp", bufs=2))

            units = []
            for pair in range(NPAIR):
                for tb in range(nt):
                    nkc = npc + (tb + 1) * (TQ // KC)
                    for kc_ in range(nkc):
                        units.append((pair, tb, kc_, nkc))

            cur_yb = [None, None]   # [ybe, ybo] accumulators of current tb

            def emit_S(u):
                pair, tb, kc_, nkc = u
                sb = sbp.tile([128, 2, TQ], F32, tag="sb", name="sb")
                for h in range(2):
                    if kc_ < npc:
                        ksrc, klo = cktT[pair], kc_ * KC
                    else:
                        ksrc, klo = kT[pair], (kc_ - npc) * KC
                    nc.tensor.matmul(
                        sb[:, h, :],
                        ksrc[h * 64:(h + 1) * 64, klo:klo + KC],
                        qT[pair][h * 64:(h + 1) * 64, tb * TQ:(tb + 1) * TQ],
                        start=True, stop=True,
                        tile_position=(h * 64, 0),
                        skip_group_check=True,
                    )
                return sb

            def emit_E(u, sb):
                pair, tb, kc_, nkc = u
                # ScalarE exp is the phase-2 pacer; offload a spread of
                # mid-prefix chunks to DVE via the Schraudolph int16 bit
                # trick (~1.6% weight error on ~15% of the key mass)
                if 4 <= kc_ <= 13 and (kc_ - 4) % 3 == 0:
                    ebi = ebp.tile([128, 2, TQ], I16, tag="eb", name="ebi")
                    nc.vector.tensor_scalar(ebi[:, :, :], sb[:, :, :],
                                            SCH_A, SCH_B, MULT, ADD)
                    return ebi.bitcast(BF16)
                eb = ebp.tile([128, 2, TQ], BF16, tag="eb", name="eb")
                nc.scalar.activation(eb[:, :, :], sb[:, :, :], EXP, scale=SCALE)
                j = kc_ - (nkc - 4)
                if j >= 0:  # causal mask on diagonal chunks
                    for h in range(2):
                        nc.vector.tensor_tensor(
                            eb[:, h, :], eb[:, h, :], maskt[:, j, :], MULT)
                return eb

            def emit_A(u, eb):
                pair, tb, kc_, nkc = u
                first, last = kc_ == 0, kc_ == nkc - 1
                if first:
                    cur_yb[0] = ybp.tile([65, TQ], F32, tag="ybe", name="ybe")
                    cur_yb[1] = ybp.tile([65, TQ], F32, tag="ybo", name="ybo")
                for h in range(2):
                    if kc_ < npc:
                        vsrc = cvt[pair][:, kc_, h, :]
                    else:
                        vsrc = vt[:, kc_ - npc, h, pair, :]
                    nc.tensor.matmul(
                        cur_yb[h][:, :], vsrc, eb[:, h, :],
                        start=first, stop=last,
                        tile_position=(0, 0),
                        skip_group_check=True,
                    )
                if last:
                    emit_tb_end(pair, tb, cur_yb[0], cur_yb[1])

            def emit_tb_end(pair, tb, ybe, ybo):
                sl = pair * nt + tb
                # evacuate y rows; head-odd goes via SBUF->SBUF DMA to reach
                # partitions 64..127 (engines cannot shift partitions).
                # Normalization itself is deferred to phase 3.
                nc.vector.tensor_copy(ysb[pair][0:64, tb, :], ybe[0:64, :])
                stage = stp.tile([65, TQ], BF16, tag="stage", name="stage")
                nc.vector.tensor_copy(stage[:, :], ybo[:, :])
                with nc.allow_low_precision(reason="recip->f32r for bcast mm"):
                    nc.vector.reciprocal(rdc[64:65, sl, :], ybe[64:65, :])
                    nc.vector.reciprocal(rdc[0:1, sl, :], ybo[0:1, :])
                nc.sync.dma_start(ysb[pair][64:128, tb, :], stage[1:65, :])

            # Two-stage software pipeline: AV lags exp by one full chunk so
            # the exp->AV semaphore is already satisfied when PE reaches the
            # AV matmuls (no per-chunk PE stall -> PE clock stays ramped).
            pend_e = None   # (unit, sb) awaiting exp
            pend_a = None   # (unit, eb) awaiting AV
            for u in units:
                sb = emit_S(u)
                if pend_a is not None:
                    emit_A(*pend_a)
                    pend_a = None
                if pend_e is not None:
                    pend_a = (pend_e[0], emit_E(*pend_e))
                pend_e = (u, sb)
            pend_a_last = (pend_e[0], emit_E(*pend_e))
            if pend_a is not None:
                emit_A(*pend_a)
            emit_A(*pend_a_last)

        nc.sync.dma_start(wpt[0][:], wp[0, :, :])
        nc.sync.dma_start(wpt[1][:], wp[1, :, :])

        # ---- phase 3: softmax normalization + output projection ----------
        with ExitStack() as ph3:
            ps_n = ph3.enter_context(tc.tile_pool(name="ps_n", bufs=2, space="PSUM"))
            ps_o = ph3.enter_context(tc.tile_pool(name="ps_o", bufs=4, space="PSUM"))
            stg = ph3.enter_context(tc.tile_pool(name="stg", bufs=4))

            def normalize(nb):
                for pair in range(NPAIR):
                    # broadcast both heads' recips across partitions via one
                    # K=65 matmul (bselt rows other than 0/64 are zero)
                    bcp = ps_n.tile([128, TQ], F32, tag="bcp", name="bcp")
                    nc.tensor.matmul(bcp[:], bselt[:, :],
                                     rdc[:, pair * nt + nb, :],
                                     start=True, stop=True,
                                     skip_group_check=True)
                    nc.vector.tensor_tensor(
                        ysb[pair][:, nb, :], ysb[pair][:, nb, :], bcp[:], MULT)

            normalize(0)
            for nb in range(nt):
                if nb + 1 < nt:
                    normalize(nb + 1)
                for mc in range(c // 128):
                    ps = ps_o.tile([128, TQ], F32, tag="ps_o", name="pso")
                    for pair in range(NPAIR):
                        nc.tensor.matmul(
                            ps[:],
                            wpt[pair][:, mc * 128:(mc + 1) * 128],
                            ysb[pair][:, nb, :],
                            start=(pair == 0), stop=(pair == NPAIR - 1),
                        )
                    ot = stg.tile([128, TQ], BF16, tag="ot", name="ot")
                    if mc % 2 == 0:
                        nc.scalar.activation(ot[:], ps[:], COPY)
                    else:
                        nc.vector.tensor_copy(ot[:], ps[:])
                    nc.sync.dma_start(
                        out_t[mc * 128:(mc + 1) * 128, nb * TQ:(nb + 1) * TQ], ot[:])

    nc.compile()
    return nc


def make_in_maps(x, W_attn, W_proj, cache_k, cache_v, n_cores=8):
    """Shard full inputs into per-core input maps (host side)."""
    b_, t_, c_ = x.shape
    h_ = cache_k.shape[1]
    d_ = c_ // h_
    p_ = cache_k.shape[2]
    hpc = h_ // (n_cores // b_)
    in_maps = []
    Wq = W_attn[:, 0 * c_:1 * c_]
    Wk = W_attn[:, 1 * c_:2 * c_]
    Wv = W_attn[:, 2 * c_:3 * c_]
    mask_np = np.zeros((128, 4, TQ), np.float32)
    for j in range(4):
        mask_np[:, j, :] = (np.arange(TQ)[None, :] >=
                            (np.arange(128)[:, None] + j * 128)).astype(np.float32)
    bsel_np = np.zeros((65, 128), np.float32)
    bsel_np[64, 0:64] = 1.0     # even-head recip lives at rdc partition 64
    bsel_np[0, 64:128] = 1.0    # odd-head recip lives at rdc partition 0
    for core in range(n_cores):
        b = core // (n_cores // b_)
        h0 = (core % (n_cores // b_)) * hpc
        heads = list(range(h0, h0 + hpc))
        cols = np.concatenate([np.arange(h * d_, (h + 1) * d_) for h in heads])
        x_t = np.ascontiguousarray(x[b].T)                       # [C, T]
        w_qk = np.ascontiguousarray(
            np.concatenate([Wq[:, cols], Wk[:, cols]], axis=1))  # [C, 512]
        # v columns grouped evens-first: [he(pair0), he(pair1), ho(pair0), ho(pair1)]
        vcols = np.concatenate(
            [np.arange(h * d_, (h + 1) * d_) for h in
             [heads[0], heads[2], heads[1], heads[3]]])
        w_v = np.ascontiguousarray(Wv[:, vcols])                 # [C, 256]
        npair = hpc // 2
        npc = p_ // KC
        ckt_np = np.zeros((npair, 128, p_), np.float32)
        cv_np = np.zeros((npair, 128, npc, 2, 65), np.float32)
        wp_np = np.zeros((npair, 128, c_), np.float32)
        for pr in range(npair):
            he, ho = heads[2 * pr], heads[2 * pr + 1]
            ckt_np[pr, 0:64] = cache_k[b, he].T
            ckt_np[pr, 64:128] = cache_k[b, ho].T
            cvr_e = cache_v[b, he].reshape(npc, KC, d_)     # [chunk, key, d]
            cvr_o = cache_v[b, ho].reshape(npc, KC, d_)
            cv_np[pr, :, :, 0, 0:64] = cvr_e.transpose(1, 0, 2)
            cv_np[pr, :, :, 0, 64] = 1.0
            cv_np[pr, :, :, 1, 1:65] = cvr_o.transpose(1, 0, 2)
            cv_np[pr, :, :, 1, 0] = 1.0
            wp_np[pr, 0:64] = W_proj[he * d_:(he + 1) * d_]
            wp_np[pr, 64:128] = W_proj[ho * d_:(ho + 1) * d_]
        in_maps.append({
            "x_t": x_t.astype(ml_dtypes.bfloat16),
            "w_qk": w_qk.astype(ml_dtypes.bfloat16),
            "w_v": w_v.astype(ml_dtypes.bfloat16),
            "ckt": ckt_np.astype(ml_dtypes.bfloat16),
            "cv": cv_np.astype(ml_dtypes.bfloat16),
            "wp": wp_np.astype(ml_dtypes.bfloat16),
            "masks": mask_np.astype(ml_dtypes.bfloat16),
            "bsel": bsel_np,
            "zrd": np.zeros((65, 8, TQ), np.float32),
        })
    return in_maps


def assemble_output(results, n_cores=8, b_=B, t_=T, c_=C):
    """Sum per-core partial out^T over head groups, transpose back."""
    out = np.zeros((b_, t_, c_), np.float32)
    per_b = n_cores // b_
    for b in range(b_):
        acc = np.zeros((c_, t_), np.float32)
        for i in range(per_b):
            acc += np.asarray(results[b * per_b + i]["out_t"], dtype=np.float32)
        out[b] = acc.T
    return out


_NC_CACHE = {}


def kernel(x, W_attn, W_proj, cache_k, cache_v):
    x = np.asarray(x, np.float32)
    W_attn = np.asarray(W_attn, np.float32)
    W_proj = np.asarray(W_proj, np.float32)
    cache_k = np.asarray(cache_k, np.float32)
    cache_v = np.asarray(cache_v, np.float32)
    if "nc" not in _NC_CACHE:
        _NC_CACHE["nc"] = build_kernel()
    nc = _NC_CACHE["nc"]
    in_maps = make_in_maps(x, W_attn, W_proj, cache_k, cache_v)
    res = run_bass_kernel_spmd(nc, in_maps, list(range(8)))
    return assemble_output(res.results)
